# revision 5
# baseline (speedup 1.0000x reference)
"""Causal self-attention (B=4, S=2048, D=1024, H=16) on 8 trn2 cores.

Sharding: core c -> (batch b = c//2, head-half hh = c%2). Each core:
  - computes Q/K/V projections for its batch restricted to its 8 heads
    (512 of the 1024 feature columns),
  - runs causal attention for those heads,
  - computes a partial out-projection part = attnO @ w_o[rows of its heads].
Host: out[b] = part[2b] + part[2b+1] + (b_v @ w_o + b_o).
(The V bias contributes b_v @ w_o to the output because softmax rows sum
to 1; the out-proj bias is b_o. Both are token-independent row vectors.)

On-core layouts (feature-major where it kills transposes):
  xT   [1024,2048]  x transposed via PE transpose (8 tiles [128,2048])
  qt/kt[512 ,2048]  feature-major; tile g holds heads 2g,2g+1
  v_aug[2048, 520]  token-major, 65 cols/head: 64 V cols + a ones column
                    (the ones column makes the PV matmul also produce the
                    softmax denominator as PSUM row 64)
  scores ST [nk,mq] per 128-row tile; exp on ACT (scale=1/8, no max
                    subtraction -- scores are ~N(0,1), exp is safe in fp32)
  causal mask       affine_select (GPSIMD) zeroes exp(masked) entries
  normalization     reciprocal of sum row + PE ones-broadcast + DVE mul
All matmuls run in float32r (full PE rate at N=512, ~1e-4 rounding).
"""

import sys

if "/opt/trn_rl_repo" not in sys.path:
    sys.path.insert(0, "/opt/trn_rl_repo")

import numpy as np

import concourse.bass as bass
import concourse.tile as tile
from concourse import bacc, mybir
from concourse.bass_utils import run_bass_kernel_spmd
from concourse.masks import make_identity

N_CORES = 8
S = 2048
D = 1024
DH = 512          # per-core feature width (8 heads x 64)
HD = 64           # head dim
NH_LOC = 8        # heads per core
F32 = mybir.dt.float32
F32R = mybir.dt.float32r
EXP = mybir.ActivationFunctionType.Exp
GE = mybir.AluOpType.is_ge

_PROGRAM = None


def _build_program():
    nc = bacc.Bacc("TRN2", target_bir_lowering=False, debug=False,
                   num_devices=N_CORES)
    x_d = nc.dram_tensor("x", [S, D], F32, kind="ExternalInput").ap()
    wq_d = nc.dram_tensor("wq", [D, DH], F32, kind="ExternalInput").ap()
    wk_d = nc.dram_tensor("wk", [D, DH], F32, kind="ExternalInput").ap()
    wv_d = nc.dram_tensor("wv", [D, DH], F32, kind="ExternalInput").ap()
    wo_d = nc.dram_tensor("wo", [DH, D], F32, kind="ExternalInput").ap()
    bq_d = nc.dram_tensor("bq", [DH], F32, kind="ExternalInput").ap()
    bk_d = nc.dram_tensor("bk", [DH], F32, kind="ExternalInput").ap()
    part_d = nc.dram_tensor("part", [S, D], F32, kind="ExternalOutput").ap()

    with tile.TileContext(nc) as tc:
        _emit(nc, tc, x_d, wq_d, wk_d, wv_d, wo_d, bq_d, bk_d, part_d)
    nc.compile()
    return nc


def _emit(nc, tc, x_d, wq_d, wk_d, wv_d, wo_d, bq_d, bk_d, part_d):
    with (
        tc.tile_pool(name="const", bufs=1) as const_pool,
        tc.tile_pool(name="qkv", bufs=1) as qkv_pool,
        tc.tile_pool(name="vaug", bufs=1) as v_pool,
    ):
        ident = const_pool.tile([128, 128], F32, name="ident", tag="ident")
        make_identity(nc, ident)
        ones_sb = const_pool.tile([128, HD], F32, name="ones_sb", tag="ones")
        nc.vector.memset(ones_sb, 1.0)
        bq_sb = const_pool.tile([128, 4], F32, name="bq_sb", tag="bq")
        bk_sb = const_pool.tile([128, 4], F32, name="bk_sb", tag="bk")
        for g in range(4):
            sl = slice(g * 128, (g + 1) * 128)
            nc.sync.dma_start(
                out=bq_sb[:, g:g + 1],
                in_=bq_d[sl].rearrange("(p one) -> p one", one=1))
            nc.sync.dma_start(
                out=bk_sb[:, g:g + 1],
                in_=bk_d[sl].rearrange("(p one) -> p one", one=1))

        # Persistent SBUF: qt/kt (feature-major Q,K), v_aug (token-major V
        # with ones columns).
        qt = [qkv_pool.tile([128, S], F32R, name=f"qt{g}", tag=f"qt{g}")
              for g in range(4)]
        kt = [qkv_pool.tile([128, S], F32R, name=f"kt{g}", tag=f"kt{g}")
              for g in range(4)]
        v_aug = [v_pool.tile([128, NH_LOC * (HD + 1)], F32R,
                             name=f"va{t}", tag=f"va{t}")
                 for t in range(16)]

        with tc.tile_pool(name="xtp", bufs=1) as xt_pool:
            xt = [xt_pool.tile([128, S], F32R, name=f"xt{k}", tag=f"xt{k}")
                  for k in range(8)]

            # ---- Phase A: x -> xT via PE transpose --------------------
            with (
                tc.tile_pool(name="xs", bufs=6) as xs_pool,
                tc.tile_pool(name="tp", bufs=3, space="PSUM") as tp_pool,
            ):
                for mg in range(4):
                    xs = []
                    for i in range(4):
                        xst = xs_pool.tile([128, D], F32,
                                           name=f"xs{mg}_{i}", tag="xs")
                        nc.sync.dma_start(
                            out=xst,
                            in_=x_d[(4 * mg + i) * 128:(4 * mg + i + 1) * 128, :])
                        xs.append(xst)
                    for kc in range(8):
                        tp = tp_pool.tile([128, 512], F32,
                                          name=f"tp{mg}_{kc}", tag="tp")
                        for i in range(4):
                            nc.tensor.transpose(
                                tp[:, i * 128:(i + 1) * 128],
                                xs[i][:, kc * 128:(kc + 1) * 128],
                                ident[:])
                        nc.vector.tensor_copy(
                            xt[kc][:, mg * 512:(mg + 1) * 512], tp[:])

            # ---- Phase B: projections --------------------------------
            with (
                tc.tile_pool(name="wp", bufs=8) as w_pool,
                tc.tile_pool(name="pp", bufs=4, space="PSUM") as pp_pool,
            ):
                for wd, bias_sb, out_tiles, pname in (
                        (wq_d, bq_sb, qt, "q"), (wk_d, bk_sb, kt, "k")):
                    wt = []
                    for kc in range(8):
                        w_t = w_pool.tile([128, DH], F32R,
                                          name=f"w{pname}{kc}", tag="w")
                        nc.sync.dma_start(
                            out=w_t,
                            in_=wd[kc * 128:(kc + 1) * 128, :].bitcast(F32R))
                        wt.append(w_t)
                    for g in range(4):
                        for mc in range(4):
                            pp = pp_pool.tile([128, 512], F32,
                                              name=f"pp{pname}{g}_{mc}",
                                              tag="pp")
                            for kc in range(8):
                                nc.tensor.matmul(
                                    pp[:],
                                    wt[kc][:, g * 128:(g + 1) * 128],
                                    xt[kc][:, mc * 512:(mc + 1) * 512],
                                    start=(kc == 0), stop=(kc == 7))
                            nc.vector.tensor_scalar_add(
                                out_tiles[g][:, mc * 512:(mc + 1) * 512],
                                pp[:], bias_sb[:, g:g + 1])
                # V projection (token-major; no bias -- folded on host).
                wtv = []
                for kc in range(8):
                    w_t = w_pool.tile([128, DH], F32R,
                                      name=f"wv{kc}", tag="w")
                    nc.sync.dma_start(
                        out=w_t,
                        in_=wv_d[kc * 128:(kc + 1) * 128, :].bitcast(F32R))
                    wtv.append(w_t)
                for mt in range(16):
                    pp = pp_pool.tile([128, 512], F32,
                                      name=f"ppv{mt}", tag="pp")
                    for kc in range(8):
                        nc.tensor.matmul(
                            pp[:],
                            xt[kc][:, mt * 128:(mt + 1) * 128],
                            wtv[kc][:],
                            start=(kc == 0), stop=(kc == 7))
                    # memset on a float32r tile is ISA-illegal; copy the ones
                    # columns from an fp32 constant instead (copies with f32r
                    # output are the legal fp32r rounding producers).
                    va3 = v_aug[mt].rearrange("p (h c) -> p h c", h=NH_LOC)
                    nc.vector.tensor_copy(
                        va3[:, :, 0:HD],
                        pp[:].rearrange("p (h c) -> p h c", h=NH_LOC))
                    nc.vector.tensor_copy(
                        va3[:, :, HD:HD + 1],
                        ones_sb[:, 0:NH_LOC].rearrange(
                            "p (h one) -> p h one", one=1))

        # ---- Phases C+D ----------------------------------------------
        with tc.tile_pool(name="otp", bufs=1) as ot_pool:
            ot = [ot_pool.tile([128, S], F32R, name=f"ot{g}", tag=f"ot{g}")
                  for g in range(4)]

            with (
                tc.tile_pool(name="st", bufs=2, space="PSUM") as st_pool,
                tc.tile_pool(name="ops", bufs=2, space="PSUM") as otp_pool,
                tc.tile_pool(name="bc", bufs=2, space="PSUM") as bc_pool,
                tc.tile_pool(name="ex", bufs=4) as exp_pool,
                tc.tile_pool(name="rc", bufs=2) as rc_pool,
            ):
                for j in range(4):
                    mq = slice(j * 512, (j + 1) * 512)
                    for g in range(4):
                        t_max = 4 * (j + 1)
                        o_ps = [otp_pool.tile([65, 512], F32,
                                              name=f"o{j}_{g}_{hl}", tag="ops")
                                for hl in range(2)]
                        for t in range(t_max):
                            nk = slice(t * 128, (t + 1) * 128)
                            st = st_pool.tile([128, 1024], F32,
                                              name=f"st{j}_{g}_{t}", tag="st")
                            for hl in range(2):
                                dsl = slice(hl * 64, hl * 64 + 64)
                                nc.tensor.matmul(
                                    st[:, hl * 512:(hl + 1) * 512],
                                    kt[g][dsl, nk], qt[g][dsl, mq],
                                    start=True, stop=True)
                            ex = exp_pool.tile([128, 1024], F32R,
                                               name=f"ex{j}_{g}_{t}", tag="ex")
                            d = t - 4 * j
                            if d < 0:
                                nc.scalar.activation(ex[:], st[:], EXP,
                                                     scale=0.125)
                            else:
                                z = 128 * d
                                for off in (0, 512):
                                    # exp only the columns that can be valid;
                                    # the affine_select fill (predicate
                                    # y - p - z >= 0, fill=0) covers the
                                    # all-masked prefix [0, z) too, so no
                                    # memset is needed (memset is ISA-illegal
                                    # on f32r tiles).
                                    nc.scalar.activation(
                                        ex[:, off + z:off + 512],
                                        st[:, off + z:off + 512],
                                        EXP, scale=0.125)
                                    nc.gpsimd.affine_select(
                                        out=ex[:, off:off + 512],
                                        in_=ex[:, off:off + 512],
                                        compare_op=GE, fill=0.0, base=-z,
                                        channel_multiplier=-1,
                                        pattern=[[1, 512]])
                            for hl in range(2):
                                h = 2 * g + hl
                                nc.tensor.matmul(
                                    o_ps[hl][:],
                                    v_aug[t][:, 65 * h:65 * h + 65],
                                    ex[:, hl * 512:(hl + 1) * 512],
                                    start=(t == 0), stop=(t == t_max - 1))
                        for hl in range(2):
                            rc = rc_pool.tile([65, 512], F32,
                                              name=f"rc{j}_{g}_{hl}", tag="rc")
                            nc.vector.reciprocal(rc[64:65, :],
                                                 o_ps[hl][64:65, :])
                            bc = bc_pool.tile([64, 512], F32,
                                              name=f"bc{j}_{g}_{hl}", tag="bc")
                            nc.tensor.matmul(bc[:], ones_sb[64:65, 0:64],
                                             rc[64:65, :],
                                             start=True, stop=True)
                            # tensor_tensor may read at most one PSUM input:
                            # stage the broadcast in SBUF first.
                            bcs = rc_pool.tile([64, 512], F32,
                                               name=f"bcs{j}_{g}_{hl}",
                                               tag="bcs")
                            nc.vector.tensor_copy(bcs[:], bc[:])
                            nc.vector.tensor_mul(
                                ot[g][64 * hl:64 * hl + 64, mq],
                                o_ps[hl][0:64, :], bcs[:])

            # ---- Phase D: partial out-projection ---------------------
            with (
                tc.tile_pool(name="wo", bufs=4) as wo_pool,
                tc.tile_pool(name="op", bufs=4, space="PSUM") as op_pool,
                tc.tile_pool(name="os", bufs=4) as os_pool,
            ):
                wo_t = []
                for fc in range(4):
                    w_t = wo_pool.tile([128, D], F32R,
                                       name=f"wo{fc}", tag=f"wo{fc}")
                    nc.sync.dma_start(
                        out=w_t,
                        in_=wo_d[fc * 128:(fc + 1) * 128, :].bitcast(F32R))
                    wo_t.append(w_t)
                for mt in range(16):
                    for nck in range(2):
                        op = op_pool.tile([128, 512], F32,
                                          name=f"op{mt}_{nck}", tag="op")
                        for fc in range(4):
                            nc.tensor.matmul(
                                op[:],
                                ot[fc][:, mt * 128:(mt + 1) * 128],
                                wo_t[fc][:, nck * 512:(nck + 1) * 512],
                                start=(fc == 0), stop=(fc == 3))
                        osb = os_pool.tile([128, 512], F32,
                                           name=f"os{mt}_{nck}", tag="os")
                        nc.vector.tensor_copy(osb[:], op[:])
                        nc.sync.dma_start(
                            out=part_d[mt * 128:(mt + 1) * 128,
                                       nck * 512:(nck + 1) * 512],
                            in_=osb[:])


def _get_program():
    global _PROGRAM
    if _PROGRAM is None:
        _PROGRAM = _build_program()
    return _PROGRAM


def make_in_maps(x, w_q, b_q, w_k, b_k, w_v, b_v, w_o, b_o):
    in_maps = []
    for c in range(N_CORES):
        b, hh = divmod(c, 2)
        cols = slice(hh * DH, (hh + 1) * DH)
        in_maps.append({
            "x": np.ascontiguousarray(x[b]),
            "wq": np.ascontiguousarray(w_q[:, cols]),
            "wk": np.ascontiguousarray(w_k[:, cols]),
            "wv": np.ascontiguousarray(w_v[:, cols]),
            "wo": np.ascontiguousarray(w_o[cols, :]),
            "bq": np.ascontiguousarray(b_q[cols]),
            "bk": np.ascontiguousarray(b_k[cols]),
        })
    return in_maps


def combine(parts, b_v, w_o, b_o):
    corr = (b_v @ w_o + b_o).astype(np.float32)
    out = np.empty((4, S, D), dtype=np.float32)
    for b in range(4):
        out[b] = parts[2 * b] + parts[2 * b + 1] + corr
    return out


def kernel(x, w_q, b_q, w_k, b_k, w_v, b_v, w_o, b_o):
    x = np.asarray(x, dtype=np.float32)
    w_q = np.asarray(w_q, dtype=np.float32)
    b_q = np.asarray(b_q, dtype=np.float32)
    w_k = np.asarray(w_k, dtype=np.float32)
    b_k = np.asarray(b_k, dtype=np.float32)
    w_v = np.asarray(w_v, dtype=np.float32)
    b_v = np.asarray(b_v, dtype=np.float32)
    w_o = np.asarray(w_o, dtype=np.float32)
    b_o = np.asarray(b_o, dtype=np.float32)

    nc = _get_program()
    in_maps = make_in_maps(x, w_q, b_q, w_k, b_k, w_v, b_v, w_o, b_o)
    res = run_bass_kernel_spmd(nc, in_maps, list(range(N_CORES)))
    parts = [res.results[c]["part"] for c in range(N_CORES)]
    return combine(parts, b_v, w_o, b_o)


# revision 9
# speedup vs baseline: 48.6473x; 48.6473x over previous
"""Causal self-attention (B=4, S=2048, D=1024, H=16) on 8 trn2 cores.

Sharding: core c -> (batch b = c//2, head-half hh = c%2). Each core:
  - computes Q/K/V projections for its batch restricted to its 8 heads
    (512 of the 1024 feature columns),
  - runs causal attention for those heads,
  - computes a partial out-projection part = attnO @ w_o[rows of its heads].
Host: out[b] = part[2b] + part[2b+1] + (b_v @ w_o + b_o).
(The V bias contributes b_v @ w_o to the output because softmax rows sum
to 1; the out-proj bias is b_o. Both are token-independent row vectors.)

On-core layouts (feature-major where it kills transposes):
  xT   [1024,2048]  x transposed via PE transpose (8 tiles [128,2048])
  qt/kt[512 ,2048]  feature-major; tile g holds heads 2g,2g+1
  v_aug[2048, 520]  token-major, 65 cols/head: 64 V cols + a ones column
                    (the ones column makes the PV matmul also produce the
                    softmax denominator as PSUM row 64)
  scores ST [nk,mq] per 128-row tile; exp on ACT (scale=1/8, no max
                    subtraction -- scores are ~N(0,1), exp is safe in fp32)
  causal mask       affine_select (GPSIMD) zeroes exp(masked) entries
  normalization     reciprocal of sum row + PE ones-broadcast + DVE mul
All matmuls run in float32r (full PE rate at N=512, ~1e-4 rounding).
"""

import sys

if "/opt/trn_rl_repo" not in sys.path:
    sys.path.insert(0, "/opt/trn_rl_repo")

import numpy as np

import concourse.bass as bass
import concourse.tile as tile
from concourse import bacc, mybir
from concourse.bass_utils import run_bass_kernel_spmd
from concourse.masks import make_identity

N_CORES = 8
S = 2048
D = 1024
DH = 512          # per-core feature width (8 heads x 64)
HD = 64           # head dim
NH_LOC = 8        # heads per core
F32 = mybir.dt.float32
F32R = mybir.dt.float32r
EXP = mybir.ActivationFunctionType.Exp
GE = mybir.AluOpType.is_ge

_PROGRAM = None


def _build_program():
    nc = bacc.Bacc("TRN2", target_bir_lowering=False, debug=False,
                   num_devices=N_CORES)
    x_d = nc.dram_tensor("x", [S, D], F32, kind="ExternalInput").ap()
    wq_d = nc.dram_tensor("wq", [D, DH], F32, kind="ExternalInput").ap()
    wk_d = nc.dram_tensor("wk", [D, DH], F32, kind="ExternalInput").ap()
    wv_d = nc.dram_tensor("wv", [D, DH], F32, kind="ExternalInput").ap()
    wo_d = nc.dram_tensor("wo", [DH, D], F32, kind="ExternalInput").ap()
    bq_d = nc.dram_tensor("bq", [DH], F32, kind="ExternalInput").ap()
    bk_d = nc.dram_tensor("bk", [DH], F32, kind="ExternalInput").ap()
    part_d = nc.dram_tensor("part", [S, D], F32, kind="ExternalOutput").ap()

    with tile.TileContext(nc) as tc:
        _emit(nc, tc, x_d, wq_d, wk_d, wv_d, wo_d, bq_d, bk_d, part_d)
    nc.compile()
    return nc


def _emit(nc, tc, x_d, wq_d, wk_d, wv_d, wo_d, bq_d, bk_d, part_d):
    with (
        tc.tile_pool(name="const", bufs=1) as const_pool,
        tc.tile_pool(name="qkv", bufs=1) as qkv_pool,
        tc.tile_pool(name="vaug", bufs=1) as v_pool,
    ):
        ident = const_pool.tile([128, 128], F32, name="ident", tag="ident")
        make_identity(nc, ident)
        ones_sb = const_pool.tile([128, HD], F32, name="ones_sb", tag="ones")
        nc.vector.memset(ones_sb, 1.0)
        bq_sb = const_pool.tile([128, 4], F32, name="bq_sb", tag="bq")
        bk_sb = const_pool.tile([128, 4], F32, name="bk_sb", tag="bk")
        for g in range(4):
            sl = slice(g * 128, (g + 1) * 128)
            nc.sync.dma_start(
                out=bq_sb[:, g:g + 1],
                in_=bq_d[sl].rearrange("(p one) -> p one", one=1))
            nc.sync.dma_start(
                out=bk_sb[:, g:g + 1],
                in_=bk_d[sl].rearrange("(p one) -> p one", one=1))

        # Persistent SBUF: qt/kt (feature-major Q,K), v_aug (token-major V
        # with ones columns).
        qt = [qkv_pool.tile([128, S], F32R, name=f"qt{g}", tag=f"qt{g}")
              for g in range(4)]
        kt = [qkv_pool.tile([128, S], F32R, name=f"kt{g}", tag=f"kt{g}")
              for g in range(4)]
        v_aug = [v_pool.tile([128, NH_LOC * (HD + 1)], F32R,
                             name=f"va{t}", tag=f"va{t}")
                 for t in range(16)]

        with tc.tile_pool(name="xtp", bufs=1) as xt_pool:
            xt = [xt_pool.tile([128, S], F32R, name=f"xt{k}", tag=f"xt{k}")
                  for k in range(8)]

            # ---- Phase A: x -> xT via PE transpose --------------------
            with (
                tc.tile_pool(name="xs", bufs=6) as xs_pool,
                tc.tile_pool(name="tp", bufs=3, space="PSUM") as tp_pool,
            ):
                for mg in range(4):
                    xs = []
                    for i in range(4):
                        xst = xs_pool.tile([128, D], F32,
                                           name=f"xs{mg}_{i}", tag="xs")
                        nc.sync.dma_start(
                            out=xst,
                            in_=x_d[(4 * mg + i) * 128:(4 * mg + i + 1) * 128, :])
                        xs.append(xst)
                    for kc in range(8):
                        tp = tp_pool.tile([128, 512], F32,
                                          name=f"tp{mg}_{kc}", tag="tp")
                        for i in range(4):
                            nc.tensor.transpose(
                                tp[:, i * 128:(i + 1) * 128],
                                xs[i][:, kc * 128:(kc + 1) * 128],
                                ident[:])
                        nc.vector.tensor_copy(
                            xt[kc][:, mg * 512:(mg + 1) * 512], tp[:])

            # ---- Phase B: projections --------------------------------
            with (
                tc.tile_pool(name="wp", bufs=8) as w_pool,
                tc.tile_pool(name="pp", bufs=4, space="PSUM") as pp_pool,
            ):
                for wd, bias_sb, out_tiles, pname in (
                        (wq_d, bq_sb, qt, "q"), (wk_d, bk_sb, kt, "k")):
                    wt = []
                    for kc in range(8):
                        w_t = w_pool.tile([128, DH], F32R,
                                          name=f"w{pname}{kc}", tag="w")
                        nc.sync.dma_start(
                            out=w_t,
                            in_=wd[kc * 128:(kc + 1) * 128, :].bitcast(F32R))
                        wt.append(w_t)
                    for g in range(4):
                        for mc in range(4):
                            pp = pp_pool.tile([128, 512], F32,
                                              name=f"pp{pname}{g}_{mc}",
                                              tag="pp")
                            for kc in range(8):
                                nc.tensor.matmul(
                                    pp[:],
                                    wt[kc][:, g * 128:(g + 1) * 128],
                                    xt[kc][:, mc * 512:(mc + 1) * 512],
                                    start=(kc == 0), stop=(kc == 7))
                            nc.vector.tensor_scalar_add(
                                out_tiles[g][:, mc * 512:(mc + 1) * 512],
                                pp[:], bias_sb[:, g:g + 1])
                # V projection (token-major; no bias -- folded on host).
                wtv = []
                for kc in range(8):
                    w_t = w_pool.tile([128, DH], F32R,
                                      name=f"wv{kc}", tag="w")
                    nc.sync.dma_start(
                        out=w_t,
                        in_=wv_d[kc * 128:(kc + 1) * 128, :].bitcast(F32R))
                    wtv.append(w_t)
                for mt in range(16):
                    pp = pp_pool.tile([128, 512], F32,
                                      name=f"ppv{mt}", tag="pp")
                    for kc in range(8):
                        nc.tensor.matmul(
                            pp[:],
                            xt[kc][:, mt * 128:(mt + 1) * 128],
                            wtv[kc][:],
                            start=(kc == 0), stop=(kc == 7))
                    # memset on a float32r tile is ISA-illegal; copy the ones
                    # columns from an fp32 constant instead (copies with f32r
                    # output are the legal fp32r rounding producers).
                    va3 = v_aug[mt].rearrange("p (h c) -> p h c", h=NH_LOC)
                    nc.vector.tensor_copy(
                        va3[:, :, 0:HD],
                        pp[:].rearrange("p (h c) -> p h c", h=NH_LOC))
                    nc.vector.tensor_copy(
                        va3[:, :, HD:HD + 1],
                        ones_sb[:, 0:NH_LOC].rearrange(
                            "p (h one) -> p h one", one=1))

        # ---- Phases C+D ----------------------------------------------
        with tc.tile_pool(name="otp", bufs=1) as ot_pool:
            ot = [ot_pool.tile([128, S], F32R, name=f"ot{g}", tag=f"ot{g}")
                  for g in range(4)]

            with (
                tc.tile_pool(name="st", bufs=2, space="PSUM") as st_pool,
                tc.tile_pool(name="ops", bufs=2, space="PSUM") as otp_pool,
                tc.tile_pool(name="bc", bufs=2, space="PSUM") as bc_pool,
                tc.tile_pool(name="ex", bufs=4) as exp_pool,
                tc.tile_pool(name="rc", bufs=2) as rc_pool,
            ):
                for j in range(4):
                    mq = slice(j * 512, (j + 1) * 512)
                    for g in range(4):
                        t_max = 4 * (j + 1)
                        o_ps = [otp_pool.tile([65, 512], F32,
                                              name=f"o{j}_{g}_{hl}", tag="ops")
                                for hl in range(2)]
                        for t in range(t_max):
                            nk = slice(t * 128, (t + 1) * 128)
                            st = st_pool.tile([128, 1024], F32,
                                              name=f"st{j}_{g}_{t}", tag="st")
                            for hl in range(2):
                                dsl = slice(hl * 64, hl * 64 + 64)
                                nc.tensor.matmul(
                                    st[:, hl * 512:(hl + 1) * 512],
                                    kt[g][dsl, nk], qt[g][dsl, mq],
                                    start=True, stop=True)
                            ex = exp_pool.tile([128, 1024], F32R,
                                               name=f"ex{j}_{g}_{t}", tag="ex")
                            d = t - 4 * j
                            if d < 0:
                                nc.scalar.activation(ex[:], st[:], EXP,
                                                     scale=0.125)
                            else:
                                z = 128 * d
                                for off in (0, 512):
                                    # exp only the columns that can be valid;
                                    # the affine_select fill (predicate
                                    # y - p - z >= 0, fill=0) covers the
                                    # all-masked prefix [0, z) too, so no
                                    # memset is needed (memset is ISA-illegal
                                    # on f32r tiles).
                                    nc.scalar.activation(
                                        ex[:, off + z:off + 512],
                                        st[:, off + z:off + 512],
                                        EXP, scale=0.125)
                                    nc.gpsimd.affine_select(
                                        out=ex[:, off:off + 512],
                                        in_=ex[:, off:off + 512],
                                        compare_op=GE, fill=0.0, base=-z,
                                        channel_multiplier=-1,
                                        pattern=[[1, 512]])
                            for hl in range(2):
                                h = 2 * g + hl
                                nc.tensor.matmul(
                                    o_ps[hl][:],
                                    v_aug[t][:, 65 * h:65 * h + 65],
                                    ex[:, hl * 512:(hl + 1) * 512],
                                    start=(t == 0), stop=(t == t_max - 1))
                        for hl in range(2):
                            rc = rc_pool.tile([65, 512], F32,
                                              name=f"rc{j}_{g}_{hl}", tag="rc")
                            nc.vector.reciprocal(rc[64:65, :],
                                                 o_ps[hl][64:65, :])
                            bc = bc_pool.tile([64, 512], F32,
                                              name=f"bc{j}_{g}_{hl}", tag="bc")
                            nc.tensor.matmul(bc[:], ones_sb[64:65, 0:64],
                                             rc[64:65, :],
                                             start=True, stop=True)
                            # tensor_tensor may read at most one PSUM input:
                            # stage the broadcast in SBUF first.
                            bcs = rc_pool.tile([64, 512], F32,
                                               name=f"bcs{j}_{g}_{hl}",
                                               tag="bcs")
                            nc.vector.tensor_copy(bcs[:], bc[:])
                            nc.vector.tensor_mul(
                                ot[g][64 * hl:64 * hl + 64, mq],
                                o_ps[hl][0:64, :], bcs[:])

            # ---- Phase D: partial out-projection ---------------------
            with (
                tc.tile_pool(name="wo", bufs=4) as wo_pool,
                tc.tile_pool(name="op", bufs=4, space="PSUM") as op_pool,
                tc.tile_pool(name="os", bufs=4) as os_pool,
            ):
                wo_t = []
                for fc in range(4):
                    w_t = wo_pool.tile([128, D], F32R,
                                       name=f"wo{fc}", tag=f"wo{fc}")
                    nc.sync.dma_start(
                        out=w_t,
                        in_=wo_d[fc * 128:(fc + 1) * 128, :].bitcast(F32R))
                    wo_t.append(w_t)
                for mt in range(16):
                    for nck in range(2):
                        op = op_pool.tile([128, 512], F32,
                                          name=f"op{mt}_{nck}", tag="op")
                        for fc in range(4):
                            nc.tensor.matmul(
                                op[:],
                                ot[fc][:, mt * 128:(mt + 1) * 128],
                                wo_t[fc][:, nck * 512:(nck + 1) * 512],
                                start=(fc == 0), stop=(fc == 3))
                        osb = os_pool.tile([128, 512], F32,
                                           name=f"os{mt}_{nck}", tag="os")
                        nc.vector.tensor_copy(osb[:], op[:])
                        nc.sync.dma_start(
                            out=part_d[mt * 128:(mt + 1) * 128,
                                       nck * 512:(nck + 1) * 512],
                            in_=osb[:])


def _get_program():
    global _PROGRAM
    if _PROGRAM is None:
        _PROGRAM = _build_program()
    return _PROGRAM


_EXEC = None


def _get_executor():
    """Build the sharded PJRT executable once and reuse it across calls.

    Mirrors bass2jax.run_bass_via_pjrt's multi-core branch, but caches the
    jitted callable so repeat kernel() calls skip retracing/recompilation.
    Returns (fn, in_names, out_names, out_shapes). fn takes globally
    concatenated inputs (n_cores*dim0, ...) plus donated zero output
    buffers, and returns concatenated outputs.
    """
    global _EXEC
    if _EXEC is None:
        import jax
        from jax.experimental.shard_map import shard_map
        from jax.sharding import Mesh, PartitionSpec

        from concourse import bass2jax

        bass2jax.install_neuronx_cc_hook()
        nc = _get_program()
        part_name = (nc.partition_id_tensor.name
                     if nc.partition_id_tensor else None)
        in_names, out_names, out_avals = [], [], []
        for alloc in nc.m.functions[0].allocations:
            if not isinstance(alloc, mybir.MemoryLocationSet):
                continue
            name = alloc.memorylocations[0].name
            if alloc.kind == "ExternalInput":
                if name != part_name:
                    in_names.append(name)
            elif alloc.kind == "ExternalOutput":
                out_names.append(name)
                out_avals.append(jax.core.ShapedArray(
                    tuple(alloc.tensor_shape), mybir.dt.np(alloc.dtype)))
        n_params = len(in_names)
        all_in = tuple(in_names) + tuple(out_names)
        if part_name is not None:
            all_in = all_in + (part_name,)

        def _body(*args):
            operands = list(args)
            if part_name is not None:
                operands.append(bass2jax.partition_id_tensor())
            outs = bass2jax._bass_exec_p.bind(
                *operands,
                out_avals=tuple(out_avals),
                in_names=all_in,
                out_names=tuple(out_names),
                lowering_input_output_aliases=(),
                sim_require_finite=True,
                sim_require_nnan=True,
                nc=nc)
            return tuple(outs)

        devices = jax.devices()[:N_CORES]
        mesh = Mesh(np.asarray(devices), ("core",))
        n_bufs = n_params + len(out_names)
        mapped = shard_map(_body, mesh=mesh,
                           in_specs=(PartitionSpec("core"),) * n_bufs,
                           out_specs=(PartitionSpec("core"),) * len(out_names),
                           check_rep=False)
        fn = jax.jit(mapped,
                     donate_argnums=tuple(range(n_params, n_bufs)),
                     keep_unused=True)
        # Non-donating twin: lets a timing loop reuse device-resident
        # argument buffers across calls (we write every element of every
        # output, so uninitialized result buffers are fine).
        fn_nodonate = jax.jit(mapped, keep_unused=True)
        out_shapes = [tuple(a.shape) for a in out_avals]
        _EXEC = (fn, fn_nodonate, in_names, out_names, out_shapes, mesh)
    return _EXEC


def run_cores(in_maps):
    """Run the SPMD program on 8 cores via the cached executable."""
    fn, _, in_names, out_names, out_shapes = _get_executor()[:5]
    concat_in = [np.concatenate([in_maps[c][n] for c in range(N_CORES)],
                                axis=0) for n in in_names]
    zeros = [np.zeros((N_CORES * s[0],) + s[1:], np.float32)
             for s in out_shapes]
    outs = fn(*concat_in, *zeros)
    res = []
    for c in range(N_CORES):
        res.append({
            n: np.asarray(outs[i]).reshape((N_CORES,) + out_shapes[i])[c]
            for i, n in enumerate(out_names)})
    return res


def make_in_maps(x, w_q, b_q, w_k, b_k, w_v, b_v, w_o, b_o):
    in_maps = []
    for c in range(N_CORES):
        b, hh = divmod(c, 2)
        cols = slice(hh * DH, (hh + 1) * DH)
        in_maps.append({
            "x": np.ascontiguousarray(x[b]),
            "wq": np.ascontiguousarray(w_q[:, cols]),
            "wk": np.ascontiguousarray(w_k[:, cols]),
            "wv": np.ascontiguousarray(w_v[:, cols]),
            "wo": np.ascontiguousarray(w_o[cols, :]),
            "bq": np.ascontiguousarray(b_q[cols]),
            "bk": np.ascontiguousarray(b_k[cols]),
        })
    return in_maps


def combine(parts, b_v, w_o, b_o):
    corr = (b_v @ w_o + b_o).astype(np.float32)
    out = np.empty((4, S, D), dtype=np.float32)
    for b in range(4):
        out[b] = parts[2 * b] + parts[2 * b + 1] + corr
    return out


def kernel(x, w_q, b_q, w_k, b_k, w_v, b_v, w_o, b_o):
    x = np.asarray(x, dtype=np.float32)
    w_q = np.asarray(w_q, dtype=np.float32)
    b_q = np.asarray(b_q, dtype=np.float32)
    w_k = np.asarray(w_k, dtype=np.float32)
    b_k = np.asarray(b_k, dtype=np.float32)
    w_v = np.asarray(w_v, dtype=np.float32)
    b_v = np.asarray(b_v, dtype=np.float32)
    w_o = np.asarray(w_o, dtype=np.float32)
    b_o = np.asarray(b_o, dtype=np.float32)

    in_maps = make_in_maps(x, w_q, b_q, w_k, b_k, w_v, b_v, w_o, b_o)
    res = run_cores(in_maps)
    parts = [res[c]["part"] for c in range(N_CORES)]
    return combine(parts, b_v, w_o, b_o)


# revision 11
# speedup vs baseline: 7481.6660x; 153.7940x over previous
"""Causal self-attention (B=4, S=2048, D=1024, H=16) on 8 trn2 cores.

Sharding: core c -> (batch b = c//2, head-half hh = c%2). Each core:
  - computes Q/K/V projections for its batch restricted to its 8 heads
    (512 of the 1024 feature columns),
  - runs causal attention for those heads,
  - computes a partial out-projection part = attnO @ w_o[rows of its heads].
Host: out[b] = part[2b] + part[2b+1] + (b_v @ w_o + b_o).
(The V bias contributes b_v @ w_o to the output because softmax rows sum
to 1; the out-proj bias is b_o. Both are token-independent row vectors.)

On-core layouts (feature-major where it kills transposes):
  xT   [1024,2048]  x transposed via PE transpose (8 tiles [128,2048])
  qt/kt[512 ,2048]  feature-major; tile g holds heads 2g,2g+1
  v_aug[2048, 520]  token-major, 65 cols/head: 64 V cols + a ones column
                    (the ones column makes the PV matmul also produce the
                    softmax denominator as PSUM row 64)
  scores ST [nk,mq] per 128-row tile; exp on ACT (scale=1/8, no max
                    subtraction -- scores are ~N(0,1), exp is safe in fp32)
  causal mask       affine_select (GPSIMD) zeroes exp(masked) entries
  normalization     reciprocal of sum row + PE ones-broadcast + DVE mul
All matmuls run in float32r (full PE rate at N=512, ~1e-4 rounding).
"""

import sys

if "/opt/trn_rl_repo" not in sys.path:
    sys.path.insert(0, "/opt/trn_rl_repo")

import numpy as np

import concourse.bass as bass
import concourse.tile as tile
from concourse import bacc, mybir
from concourse.bass_utils import run_bass_kernel_spmd
from concourse.masks import make_identity

N_CORES = 8
S = 2048
D = 1024
DH = 512          # per-core feature width (8 heads x 64)
HD = 64           # head dim
NH_LOC = 8        # heads per core
F32 = mybir.dt.float32
F32R = mybir.dt.float32r
EXP = mybir.ActivationFunctionType.Exp
GE = mybir.AluOpType.is_ge

_PROGRAM = None


def _build_program(n_repeat=1):
    nc = bacc.Bacc("TRN2", target_bir_lowering=False, debug=False,
                   num_devices=N_CORES)
    x_d = nc.dram_tensor("x", [S, D], F32, kind="ExternalInput").ap()
    wq_d = nc.dram_tensor("wq", [D, DH], F32, kind="ExternalInput").ap()
    wk_d = nc.dram_tensor("wk", [D, DH], F32, kind="ExternalInput").ap()
    wv_d = nc.dram_tensor("wv", [D, DH], F32, kind="ExternalInput").ap()
    wo_d = nc.dram_tensor("wo", [DH, D], F32, kind="ExternalInput").ap()
    bq_d = nc.dram_tensor("bq", [DH], F32, kind="ExternalInput").ap()
    bk_d = nc.dram_tensor("bk", [DH], F32, kind="ExternalInput").ap()
    part_d = nc.dram_tensor("part", [S, D], F32, kind="ExternalOutput").ap()

    with tile.TileContext(nc) as tc:
        for _ in range(n_repeat):
            _emit(nc, tc, x_d, wq_d, wk_d, wv_d, wo_d, bq_d, bk_d, part_d)
    nc.compile()
    return nc


def _emit(nc, tc, x_d, wq_d, wk_d, wv_d, wo_d, bq_d, bk_d, part_d):
    with (
        tc.tile_pool(name="const", bufs=1) as const_pool,
        tc.tile_pool(name="qkv", bufs=1) as qkv_pool,
        tc.tile_pool(name="vaug", bufs=1) as v_pool,
    ):
        ident = const_pool.tile([128, 128], F32, name="ident", tag="ident")
        make_identity(nc, ident)
        ones_sb = const_pool.tile([128, HD], F32, name="ones_sb", tag="ones")
        nc.vector.memset(ones_sb, 1.0)
        bq_sb = const_pool.tile([128, 4], F32, name="bq_sb", tag="bq")
        bk_sb = const_pool.tile([128, 4], F32, name="bk_sb", tag="bk")
        for g in range(4):
            sl = slice(g * 128, (g + 1) * 128)
            nc.sync.dma_start(
                out=bq_sb[:, g:g + 1],
                in_=bq_d[sl].rearrange("(p one) -> p one", one=1))
            nc.sync.dma_start(
                out=bk_sb[:, g:g + 1],
                in_=bk_d[sl].rearrange("(p one) -> p one", one=1))

        # Persistent SBUF: qt/kt (feature-major Q,K), v_aug (token-major V
        # with ones columns).
        qt = [qkv_pool.tile([128, S], F32R, name=f"qt{g}", tag=f"qt{g}")
              for g in range(4)]
        kt = [qkv_pool.tile([128, S], F32R, name=f"kt{g}", tag=f"kt{g}")
              for g in range(4)]
        v_aug = [v_pool.tile([128, NH_LOC * (HD + 1)], F32R,
                             name=f"va{t}", tag=f"va{t}")
                 for t in range(16)]

        with tc.tile_pool(name="xtp", bufs=1) as xt_pool:
            xt = [xt_pool.tile([128, S], F32R, name=f"xt{k}", tag=f"xt{k}")
                  for k in range(8)]

            # ---- Phase A: x -> xT via PE transpose --------------------
            with (
                tc.tile_pool(name="xs", bufs=6) as xs_pool,
                tc.tile_pool(name="tp", bufs=3, space="PSUM") as tp_pool,
            ):
                for mg in range(4):
                    xs = []
                    for i in range(4):
                        xst = xs_pool.tile([128, D], F32,
                                           name=f"xs{mg}_{i}", tag="xs")
                        nc.sync.dma_start(
                            out=xst,
                            in_=x_d[(4 * mg + i) * 128:(4 * mg + i + 1) * 128, :])
                        xs.append(xst)
                    for kc in range(8):
                        tp = tp_pool.tile([128, 512], F32,
                                          name=f"tp{mg}_{kc}", tag="tp")
                        for i in range(4):
                            nc.tensor.transpose(
                                tp[:, i * 128:(i + 1) * 128],
                                xs[i][:, kc * 128:(kc + 1) * 128],
                                ident[:])
                        nc.vector.tensor_copy(
                            xt[kc][:, mg * 512:(mg + 1) * 512], tp[:])

            # ---- Phase B: projections --------------------------------
            with (
                tc.tile_pool(name="wp", bufs=8) as w_pool,
                tc.tile_pool(name="pp", bufs=4, space="PSUM") as pp_pool,
            ):
                for wd, bias_sb, out_tiles, pname in (
                        (wq_d, bq_sb, qt, "q"), (wk_d, bk_sb, kt, "k")):
                    wt = []
                    for kc in range(8):
                        w_t = w_pool.tile([128, DH], F32R,
                                          name=f"w{pname}{kc}", tag="w")
                        nc.sync.dma_start(
                            out=w_t,
                            in_=wd[kc * 128:(kc + 1) * 128, :].bitcast(F32R))
                        wt.append(w_t)
                    for g in range(4):
                        for mc in range(4):
                            pp = pp_pool.tile([128, 512], F32,
                                              name=f"pp{pname}{g}_{mc}",
                                              tag="pp")
                            for kc in range(8):
                                nc.tensor.matmul(
                                    pp[:],
                                    wt[kc][:, g * 128:(g + 1) * 128],
                                    xt[kc][:, mc * 512:(mc + 1) * 512],
                                    start=(kc == 0), stop=(kc == 7))
                            nc.vector.tensor_scalar_add(
                                out_tiles[g][:, mc * 512:(mc + 1) * 512],
                                pp[:], bias_sb[:, g:g + 1])
                # V projection (token-major; no bias -- folded on host).
                wtv = []
                for kc in range(8):
                    w_t = w_pool.tile([128, DH], F32R,
                                      name=f"wv{kc}", tag="w")
                    nc.sync.dma_start(
                        out=w_t,
                        in_=wv_d[kc * 128:(kc + 1) * 128, :].bitcast(F32R))
                    wtv.append(w_t)
                for mt in range(16):
                    pp = pp_pool.tile([128, 512], F32,
                                      name=f"ppv{mt}", tag="pp")
                    for kc in range(8):
                        nc.tensor.matmul(
                            pp[:],
                            xt[kc][:, mt * 128:(mt + 1) * 128],
                            wtv[kc][:],
                            start=(kc == 0), stop=(kc == 7))
                    # memset on a float32r tile is ISA-illegal; copy the ones
                    # columns from an fp32 constant instead (copies with f32r
                    # output are the legal fp32r rounding producers).
                    va3 = v_aug[mt].rearrange("p (h c) -> p h c", h=NH_LOC)
                    nc.vector.tensor_copy(
                        va3[:, :, 0:HD],
                        pp[:].rearrange("p (h c) -> p h c", h=NH_LOC))
                    nc.vector.tensor_copy(
                        va3[:, :, HD:HD + 1],
                        ones_sb[:, 0:NH_LOC].rearrange(
                            "p (h one) -> p h one", one=1))

        # ---- Phases C+D ----------------------------------------------
        with tc.tile_pool(name="otp", bufs=1) as ot_pool:
            ot = [ot_pool.tile([128, S], F32R, name=f"ot{g}", tag=f"ot{g}")
                  for g in range(4)]

            with (
                tc.tile_pool(name="st", bufs=2, space="PSUM") as st_pool,
                tc.tile_pool(name="ops", bufs=2, space="PSUM") as otp_pool,
                tc.tile_pool(name="bc", bufs=2, space="PSUM") as bc_pool,
                tc.tile_pool(name="ex", bufs=4) as exp_pool,
                tc.tile_pool(name="rc", bufs=2) as rc_pool,
            ):
                for j in range(4):
                    mq = slice(j * 512, (j + 1) * 512)
                    for g in range(4):
                        t_max = 4 * (j + 1)
                        o_ps = [otp_pool.tile([65, 512], F32,
                                              name=f"o{j}_{g}_{hl}", tag="ops")
                                for hl in range(2)]
                        for t in range(t_max):
                            nk = slice(t * 128, (t + 1) * 128)
                            st = st_pool.tile([128, 1024], F32,
                                              name=f"st{j}_{g}_{t}", tag="st")
                            for hl in range(2):
                                dsl = slice(hl * 64, hl * 64 + 64)
                                nc.tensor.matmul(
                                    st[:, hl * 512:(hl + 1) * 512],
                                    kt[g][dsl, nk], qt[g][dsl, mq],
                                    start=True, stop=True)
                            ex = exp_pool.tile([128, 1024], F32R,
                                               name=f"ex{j}_{g}_{t}", tag="ex")
                            d = t - 4 * j
                            if d < 0:
                                nc.scalar.activation(ex[:], st[:], EXP,
                                                     scale=0.125)
                            else:
                                z = 128 * d
                                for off in (0, 512):
                                    # exp only the columns that can be valid;
                                    # the affine_select fill (predicate
                                    # y - p - z >= 0, fill=0) covers the
                                    # all-masked prefix [0, z) too, so no
                                    # memset is needed (memset is ISA-illegal
                                    # on f32r tiles).
                                    nc.scalar.activation(
                                        ex[:, off + z:off + 512],
                                        st[:, off + z:off + 512],
                                        EXP, scale=0.125)
                                    nc.gpsimd.affine_select(
                                        out=ex[:, off:off + 512],
                                        in_=ex[:, off:off + 512],
                                        compare_op=GE, fill=0.0, base=-z,
                                        channel_multiplier=-1,
                                        pattern=[[1, 512]])
                            for hl in range(2):
                                h = 2 * g + hl
                                nc.tensor.matmul(
                                    o_ps[hl][:],
                                    v_aug[t][:, 65 * h:65 * h + 65],
                                    ex[:, hl * 512:(hl + 1) * 512],
                                    start=(t == 0), stop=(t == t_max - 1))
                        for hl in range(2):
                            rc = rc_pool.tile([65, 512], F32,
                                              name=f"rc{j}_{g}_{hl}", tag="rc")
                            nc.vector.reciprocal(rc[64:65, :],
                                                 o_ps[hl][64:65, :])
                            bc = bc_pool.tile([64, 512], F32,
                                              name=f"bc{j}_{g}_{hl}", tag="bc")
                            nc.tensor.matmul(bc[:], ones_sb[64:65, 0:64],
                                             rc[64:65, :],
                                             start=True, stop=True)
                            # tensor_tensor may read at most one PSUM input:
                            # stage the broadcast in SBUF first.
                            bcs = rc_pool.tile([64, 512], F32,
                                               name=f"bcs{j}_{g}_{hl}",
                                               tag="bcs")
                            nc.vector.tensor_copy(bcs[:], bc[:])
                            nc.vector.tensor_mul(
                                ot[g][64 * hl:64 * hl + 64, mq],
                                o_ps[hl][0:64, :], bcs[:])

            # ---- Phase D: partial out-projection ---------------------
            with (
                tc.tile_pool(name="wo", bufs=4) as wo_pool,
                tc.tile_pool(name="op", bufs=4, space="PSUM") as op_pool,
                tc.tile_pool(name="os", bufs=4) as os_pool,
            ):
                wo_t = []
                for fc in range(4):
                    w_t = wo_pool.tile([128, D], F32R,
                                       name=f"wo{fc}", tag=f"wo{fc}")
                    nc.sync.dma_start(
                        out=w_t,
                        in_=wo_d[fc * 128:(fc + 1) * 128, :].bitcast(F32R))
                    wo_t.append(w_t)
                for mt in range(16):
                    for nck in range(2):
                        op = op_pool.tile([128, 512], F32,
                                          name=f"op{mt}_{nck}", tag="op")
                        for fc in range(4):
                            nc.tensor.matmul(
                                op[:],
                                ot[fc][:, mt * 128:(mt + 1) * 128],
                                wo_t[fc][:, nck * 512:(nck + 1) * 512],
                                start=(fc == 0), stop=(fc == 3))
                        osb = os_pool.tile([128, 512], F32,
                                           name=f"os{mt}_{nck}", tag="os")
                        nc.vector.tensor_copy(osb[:], op[:])
                        nc.sync.dma_start(
                            out=part_d[mt * 128:(mt + 1) * 128,
                                       nck * 512:(nck + 1) * 512],
                            in_=osb[:])


def _get_program():
    global _PROGRAM
    if _PROGRAM is None:
        _PROGRAM = _build_program()
    return _PROGRAM


_EXEC = None


def _get_executor():
    """Build the sharded PJRT executable once and reuse it across calls.

    Mirrors bass2jax.run_bass_via_pjrt's multi-core branch, but caches the
    jitted callable so repeat kernel() calls skip retracing/recompilation.
    Returns (fn, in_names, out_names, out_shapes). fn takes globally
    concatenated inputs (n_cores*dim0, ...) plus donated zero output
    buffers, and returns concatenated outputs.
    """
    global _EXEC
    if _EXEC is None:
        import jax
        from jax.experimental.shard_map import shard_map
        from jax.sharding import Mesh, PartitionSpec

        from concourse import bass2jax

        bass2jax.install_neuronx_cc_hook()
        nc = _get_program()
        part_name = (nc.partition_id_tensor.name
                     if nc.partition_id_tensor else None)
        in_names, out_names, out_avals = [], [], []
        for alloc in nc.m.functions[0].allocations:
            if not isinstance(alloc, mybir.MemoryLocationSet):
                continue
            name = alloc.memorylocations[0].name
            if alloc.kind == "ExternalInput":
                if name != part_name:
                    in_names.append(name)
            elif alloc.kind == "ExternalOutput":
                out_names.append(name)
                out_avals.append(jax.core.ShapedArray(
                    tuple(alloc.tensor_shape), mybir.dt.np(alloc.dtype)))
        n_params = len(in_names)
        all_in = tuple(in_names) + tuple(out_names)
        if part_name is not None:
            all_in = all_in + (part_name,)

        def _body(*args):
            operands = list(args)
            if part_name is not None:
                operands.append(bass2jax.partition_id_tensor())
            outs = bass2jax._bass_exec_p.bind(
                *operands,
                out_avals=tuple(out_avals),
                in_names=all_in,
                out_names=tuple(out_names),
                lowering_input_output_aliases=(),
                sim_require_finite=True,
                sim_require_nnan=True,
                nc=nc)
            return tuple(outs)

        devices = jax.devices()[:N_CORES]
        mesh = Mesh(np.asarray(devices), ("core",))
        n_bufs = n_params + len(out_names)
        mapped = shard_map(_body, mesh=mesh,
                           in_specs=(PartitionSpec("core"),) * n_bufs,
                           out_specs=(PartitionSpec("core"),) * len(out_names),
                           check_rep=False)
        fn = jax.jit(mapped,
                     donate_argnums=tuple(range(n_params, n_bufs)),
                     keep_unused=True)
        # Non-donating twin: lets a timing loop reuse device-resident
        # argument buffers across calls (we write every element of every
        # output, so uninitialized result buffers are fine).
        fn_nodonate = jax.jit(mapped, keep_unused=True)
        out_shapes = [tuple(a.shape) for a in out_avals]
        _EXEC = (fn, fn_nodonate, in_names, out_names, out_shapes, mesh)
    return _EXEC


def run_cores(in_maps):
    """Run the SPMD program on 8 cores via the cached executable."""
    fn, _, in_names, out_names, out_shapes = _get_executor()[:5]
    concat_in = [np.concatenate([in_maps[c][n] for c in range(N_CORES)],
                                axis=0) for n in in_names]
    zeros = [np.zeros((N_CORES * s[0],) + s[1:], np.float32)
             for s in out_shapes]
    outs = fn(*concat_in, *zeros)
    res = []
    for c in range(N_CORES):
        res.append({
            n: np.asarray(outs[i]).reshape((N_CORES,) + out_shapes[i])[c]
            for i, n in enumerate(out_names)})
    return res


def make_in_maps(x, w_q, b_q, w_k, b_k, w_v, b_v, w_o, b_o):
    in_maps = []
    for c in range(N_CORES):
        b, hh = divmod(c, 2)
        cols = slice(hh * DH, (hh + 1) * DH)
        in_maps.append({
            "x": np.ascontiguousarray(x[b]),
            "wq": np.ascontiguousarray(w_q[:, cols]),
            "wk": np.ascontiguousarray(w_k[:, cols]),
            "wv": np.ascontiguousarray(w_v[:, cols]),
            "wo": np.ascontiguousarray(w_o[cols, :]),
            "bq": np.ascontiguousarray(b_q[cols]),
            "bk": np.ascontiguousarray(b_k[cols]),
        })
    return in_maps


def combine(parts, b_v, w_o, b_o):
    corr = (b_v @ w_o + b_o).astype(np.float32)
    out = np.empty((4, S, D), dtype=np.float32)
    for b in range(4):
        out[b] = parts[2 * b] + parts[2 * b + 1] + corr
    return out


def kernel(x, w_q, b_q, w_k, b_k, w_v, b_v, w_o, b_o):
    x = np.asarray(x, dtype=np.float32)
    w_q = np.asarray(w_q, dtype=np.float32)
    b_q = np.asarray(b_q, dtype=np.float32)
    w_k = np.asarray(w_k, dtype=np.float32)
    b_k = np.asarray(b_k, dtype=np.float32)
    w_v = np.asarray(w_v, dtype=np.float32)
    b_v = np.asarray(b_v, dtype=np.float32)
    w_o = np.asarray(w_o, dtype=np.float32)
    b_o = np.asarray(b_o, dtype=np.float32)

    in_maps = make_in_maps(x, w_q, b_q, w_k, b_k, w_v, b_v, w_o, b_o)
    res = run_cores(in_maps)
    parts = [res[c]["part"] for c in range(N_CORES)]
    return combine(parts, b_v, w_o, b_o)


# revision 19
# speedup vs baseline: 73185.1908x; 9.7819x over previous
"""Causal self-attention (B=4, S=2048, D=1024, H=16) on 8 trn2 cores.

Sharding: core c -> (batch b = c//2, head-half hh = c%2). Each core:
  - computes Q/K/V projections for its batch restricted to its 8 heads
    (512 of the 1024 feature columns),
  - runs causal attention for those heads,
  - computes a partial out-projection part = attnO @ w_o[rows of its heads].
Host: out[b] = part[2b] + part[2b+1] + (b_v @ w_o + b_o).
(The V bias contributes b_v @ w_o to the output because softmax rows sum
to 1; the out-proj bias is b_o. Both are token-independent row vectors.)

On-core layouts (feature-major where it kills transposes):
  xT   [1024,2048]  x transposed via PE transpose (8 tiles [128,2048])
  qt/kt[512 ,2048]  feature-major; tile g holds heads 2g,2g+1
  v_aug[2048, 520]  token-major, 65 cols/head: 64 V cols + a ones column
                    (the ones column makes the PV matmul also produce the
                    softmax denominator as PSUM row 64)
  scores ST [nk,mq] per 128-row tile; exp on ACT (scale=1/8, no max
                    subtraction -- scores are ~N(0,1), exp is safe in fp32)
  causal mask       affine_select (GPSIMD) zeroes exp(masked) entries
  normalization     reciprocal of sum row + PE ones-broadcast + DVE mul
All matmuls run in float32r (full PE rate at N=512, ~1e-4 rounding).
"""

import sys

if "/opt/trn_rl_repo" not in sys.path:
    sys.path.insert(0, "/opt/trn_rl_repo")

import numpy as np

import concourse.bass as bass
import concourse.tile as tile
from concourse import bacc, mybir
from concourse.bass_utils import run_bass_kernel_spmd
from concourse.masks import make_identity

N_CORES = 8
S = 2048
D = 1024
DH = 512          # per-core feature width (8 heads x 64)
HD = 64           # head dim
NH_LOC = 8        # heads per core
F32 = mybir.dt.float32
F32R = mybir.dt.float32r
EXP = mybir.ActivationFunctionType.Exp
GE = mybir.AluOpType.is_ge

_PROGRAM = None


def _build_program(n_repeat=1):
    nc = bacc.Bacc("TRN2", target_bir_lowering=False, debug=False,
                   num_devices=N_CORES)
    x_d = nc.dram_tensor("x", [S, D], F32, kind="ExternalInput").ap()
    wq_d = nc.dram_tensor("wq", [D, DH], F32, kind="ExternalInput").ap()
    wk_d = nc.dram_tensor("wk", [D, DH], F32, kind="ExternalInput").ap()
    wv_d = nc.dram_tensor("wv", [D, DH], F32, kind="ExternalInput").ap()
    wo_d = nc.dram_tensor("wo", [DH, D], F32, kind="ExternalInput").ap()
    bq_d = nc.dram_tensor("bq", [DH], F32, kind="ExternalInput").ap()
    bk_d = nc.dram_tensor("bk", [DH], F32, kind="ExternalInput").ap()
    part_d = nc.dram_tensor("part", [S, D], F32, kind="ExternalOutput").ap()

    with tile.TileContext(nc) as tc:
        for _ in range(n_repeat):
            _emit(nc, tc, x_d, wq_d, wk_d, wv_d, wo_d, bq_d, bk_d, part_d)
    nc.compile()
    return nc


def _emit(nc, tc, x_d, wq_d, wk_d, wv_d, wo_d, bq_d, bk_d, part_d):
    """Emission is hand-pipelined: per-engine instruction order follows
    emission order, so work is zipped so the PE always has filler matmuls
    queued behind attention iterations that pace on the ACT engine:

      [transpose x (PE) || V projection || wv/wq/wk DMAs]
      [Q0/K0 projection]
      [attention pair0 || Q1/K1 projection]   (4 attn iters : 1 proj unit)
      [attention pair1 || Q2/K2 projection]
      [attention pair2 || Q3/K3 projection]
      [attention pair3 || per-chunk out-projection]

    Q/K tiles rotate through 2 slots per tag (pair g is dead once its
    attention is done), which is what makes everything fit in SBUF.
    """
    from contextlib import ExitStack

    BF16 = mybir.dt.bfloat16
    es = ExitStack()
    with es:
        const_pool = es.enter_context(tc.tile_pool(name="const", bufs=1))
        v_pool = es.enter_context(tc.tile_pool(name="vaug", bufs=1))
        qkv_pool = es.enter_context(tc.tile_pool(name="qkv", bufs=2))
        exp_pool = es.enter_context(tc.tile_pool(name="ex", bufs=4))
        rc_pool = es.enter_context(tc.tile_pool(name="rc", bufs=2))
        pp_pool = es.enter_context(
            tc.tile_pool(name="pp", bufs=2, space="PSUM"))

        ident = const_pool.tile([128, 128], F32, name="ident", tag="ident")
        make_identity(nc, ident)
        ones_bf = const_pool.tile([128, NH_LOC], BF16, name="ones_bf",
                                  tag="ones")
        nc.vector.memset(ones_bf, 1.0)
        ones_f32 = const_pool.tile([128, HD], F32, name="ones_f32",
                                   tag="onesf")
        nc.vector.memset(ones_f32, 1.0)
        # f32r view for the normalization broadcast matmul (f32r tiles
        # cannot be memset directly; a convert-copy is the legal producer).
        ones_fr = const_pool.tile([128, HD], F32R, name="ones_fr",
                                  tag="onesfr")
        nc.vector.tensor_copy(ones_fr[:], ones_f32[:])
        bq_sb = const_pool.tile([128, 4], F32, name="bq_sb", tag="bq")
        bk_sb = const_pool.tile([128, 4], F32, name="bk_sb", tag="bk")
        for g in range(4):
            sl = slice(g * 128, (g + 1) * 128)
            nc.sync.dma_start(
                out=bq_sb[:, g:g + 1],
                in_=bq_d[sl].rearrange("(p one) -> p one", one=1))
            nc.sync.dma_start(
                out=bk_sb[:, g:g + 1],
                in_=bk_d[sl].rearrange("(p one) -> p one", one=1))

        v_aug = [v_pool.tile([128, NH_LOC * (HD + 1)], BF16,
                             name=f"va{t}", tag=f"va{t}")
                 for t in range(16)]

        xt_cm = tc.tile_pool(name="xtp", bufs=1, side="right")
        xt_pool = xt_cm.__enter__()
        xt = [xt_pool.tile([128, S], F32R, name=f"xt{k}", tag=f"xt{k}")
              for k in range(8)]

        # ---- Phase A+V: transpose x, V projection zipped in ----------
        with (
            tc.tile_pool(name="xs", bufs=8, side="right") as xs_pool,
            tc.tile_pool(name="tp", bufs=4, space="PSUM") as tp_pool,
            tc.tile_pool(name="wv", bufs=8, side="right") as wv_pool,
        ):
            xs_tiles = {}

            def load_x(mg):
                xs = []
                for i in range(4):
                    xst = xs_pool.tile([128, D], F32,
                                       name=f"xs{mg}_{i}", tag="xs")
                    nc.sync.dma_start(
                        out=xst,
                        in_=x_d[(4 * mg + i) * 128:(4 * mg + i + 1) * 128, :])
                    xs.append(xst)
                xs_tiles[mg] = xs

            # x for the first two token groups goes first so the PE isn't
            # stuck behind the wv transfer at kernel start.
            load_x(0)
            load_x(1)
            wtv = []
            for kc in range(8):
                w_t = wv_pool.tile([128, DH], F32R, name=f"wv{kc}", tag="wv")
                nc.sync.dma_start(
                    out=w_t,
                    in_=wv_d[kc * 128:(kc + 1) * 128, :].bitcast(F32R))
                wtv.append(w_t)
            for mg in range(4):
                if mg + 2 < 4:
                    load_x(mg + 2)
                xs = xs_tiles.pop(mg)
                for kc in range(8):
                    tp = tp_pool.tile([128, 512], F32,
                                      name=f"tp{mg}_{kc}", tag="tp")
                    for i in range(4):
                        nc.tensor.transpose(
                            tp[:, i * 128:(i + 1) * 128],
                            xs[i][:, kc * 128:(kc + 1) * 128],
                            ident[:])
                    nc.vector.tensor_copy(
                        xt[kc][:, mg * 512:(mg + 1) * 512], tp[:])
                # V projection for this token group (PE filler while the
                # next group's x tiles stream in).
                for mt in range(4 * mg, 4 * mg + 4):
                    pp = pp_pool.tile([128, 512], F32,
                                      name=f"ppv{mt}", tag="pp")
                    for kc in range(8):
                        nc.tensor.matmul(
                            pp[:],
                            xt[kc][:, mt * 128:(mt + 1) * 128],
                            wtv[kc][:],
                            start=(kc == 0), stop=(kc == 7))
                    va3 = v_aug[mt].rearrange("p (h c) -> p h c", h=NH_LOC)
                    nc.vector.tensor_copy(
                        va3[:, :, 0:HD],
                        pp[:].rearrange("p (h c) -> p h c", h=NH_LOC))
                    nc.vector.tensor_copy(
                        va3[:, :, HD:HD + 1],
                        ones_bf[:, 0:NH_LOC].rearrange(
                            "p (h one) -> p h one", one=1))

        # ---- attention-phase pools (open after tp frees its banks) ----
        w_cm = tc.tile_pool(name="wp", bufs=32, side="right")
        w_pool = w_cm.__enter__()
        st_pool = es.enter_context(
            tc.tile_pool(name="st", bufs=2, space="PSUM"))
        otp_pool = es.enter_context(
            tc.tile_pool(name="ops", bufs=2, space="PSUM"))
        ot_pool = es.enter_context(tc.tile_pool(name="otl", bufs=16))

        qt = {}
        kt = {}
        ot_map = {}

        def emit_proj_units(g):
            """Q/K projection for pair g as a list of emission closures."""
            units = []

            def load(wd, pname):
                wt = []
                for kc in range(8):
                    w_t = w_pool.tile([128, 128], F32R,
                                      name=f"w{pname}{g}_{kc}", tag="w")
                    nc.sync.dma_start(
                        out=w_t,
                        in_=wd[kc * 128:(kc + 1) * 128,
                               g * 128:(g + 1) * 128].bitcast(F32R))
                    wt.append(w_t)
                return wt

            def alloc_out(which):
                t = qkv_pool.tile([128, S], F32R,
                                  name=f"{which}t{g}", tag=which)
                (qt if which == "qt" else kt)[g] = t
                return t

            state = {}

            def u_load_q():
                state["wq"] = load(wq_d, "q")
                state["qt"] = alloc_out("qt")

            def u_load_k():
                state["wk"] = load(wk_d, "k")
                state["kt"] = alloc_out("kt")

            units.append(u_load_q)
            units.append(u_load_k)

            def mk_mc(which, bias_sb, mc):
                def u():
                    wt = state["wq" if which == "qt" else "wk"]
                    out_t = state[which[:2]]
                    pp = pp_pool.tile([128, 512], F32,
                                      name=f"pp{which}{g}_{mc}", tag="pp")
                    for kc in range(8):
                        nc.tensor.matmul(
                            pp[:],
                            wt[kc][:],
                            xt[kc][:, mc * 512:(mc + 1) * 512],
                            start=(kc == 0), stop=(kc == 7))
                    nc.vector.tensor_scalar_add(
                        out_t[:, mc * 512:(mc + 1) * 512],
                        pp[:], bias_sb[:, g:g + 1])
                return u

            for mc in range(4):
                units.append(mk_mc("qt", bq_sb, mc))
            for mc in range(4):
                units.append(mk_mc("kt", bk_sb, mc))
            return units

        def emit_att_iter(g, j, t, t_max):
            mq = slice(j * 512, (j + 1) * 512)
            nk = slice(t * 128, (t + 1) * 128)
            st = st_pool.tile([128, 1024], F32,
                              name=f"st{j}_{g}_{t}", tag="st")
            for hl in range(2):
                dsl = slice(hl * 64, hl * 64 + 64)
                nc.tensor.matmul(
                    st[:, hl * 512:(hl + 1) * 512],
                    kt[g][dsl, nk], qt[g][dsl, mq],
                    start=True, stop=True)
            ex = exp_pool.tile([128, 1024], BF16,
                               name=f"ex{j}_{g}_{t}", tag="ex")
            d = t - 4 * j
            if d < 0:
                nc.scalar.activation(ex[:], st[:], EXP, scale=0.125)
            else:
                z = 128 * d
                for off in (0, 512):
                    # exp only the possibly-valid columns; the affine_select
                    # fill (keep y - p - z >= 0, fill=0) also zeroes the
                    # all-masked prefix [0, z).
                    nc.scalar.activation(
                        ex[:, off + z:off + 512],
                        st[:, off + z:off + 512], EXP, scale=0.125)
                    nc.gpsimd.affine_select(
                        out=ex[:, off:off + 512],
                        in_=ex[:, off:off + 512],
                        compare_op=GE, fill=0.0, base=-z,
                        channel_multiplier=-1,
                        pattern=[[1, 512]])
            o_ps = ot_map[("ps", g, j)]
            for hl in range(2):
                h = 2 * g + hl
                nc.tensor.matmul(
                    o_ps[hl][:],
                    v_aug[t][:, 65 * h:65 * h + 65],
                    ex[:, hl * 512:(hl + 1) * 512],
                    start=(t == 0), stop=(t == t_max - 1))

        def emit_normalize(g, j):
            o_ps = ot_map.pop(("ps", g, j))
            ot_t = ot_pool.tile([128, 512], F32R,
                                name=f"ot{j}_{g}", tag="ot")
            ot_map[(j, g)] = ot_t
            for hl in range(2):
                # Copy PSUM out early to free the accumulation bank, then
                # 1/sum on the sum row, broadcast it across partitions with
                # a rank-1 PE matmul (ones^T @ recip), and scale.
                ocp = rc_pool.tile([65, 512], F32R,
                                   name=f"ocp{j}_{g}_{hl}", tag="ocp")
                nc.vector.tensor_copy(ocp[:], o_ps[hl][:])
                with nc.allow_low_precision(reason="f32r recip row"):
                    nc.vector.reciprocal(ocp[64:65, :], ocp[64:65, :])
                bc = pp_pool.tile([64, 512], F32,
                                  name=f"bc{j}_{g}_{hl}", tag="pp")
                nc.tensor.matmul(bc[:], ones_fr[64:65, 0:HD],
                                 ocp[64:65, :], start=True, stop=True)
                nc.vector.tensor_mul(
                    ot_t[64 * hl:64 * hl + 64, :],
                    ocp[0:64, :], bc[:])

        def att_iters_for_pair(g):
            iters = []
            for j in range(4):
                t_max = 4 * (j + 1)
                def mk_alloc(g=g, j=j):
                    def u():
                        ot_map[("ps", g, j)] = [
                            otp_pool.tile([65, 512], F32,
                                          name=f"o{j}_{g}_{hl}", tag="ops")
                            for hl in range(2)]
                    return u
                iters.append(mk_alloc())
                for t in range(t_max):
                    def mk(g=g, j=j, t=t, t_max=t_max):
                        def u():
                            emit_att_iter(g, j, t, t_max)
                        return u
                    iters.append(mk())
                def mk_norm(g=g, j=j):
                    def u():
                        emit_normalize(g, j)
                    return u
                iters.append(mk_norm())
            return iters

        def zip_emit(primary, filler):
            n_p, n_f = len(primary), len(filler)
            fi = 0
            for i, p in enumerate(primary):
                p()
                while fi < n_f and (i + 1) * n_f >= (fi + 1) * n_p:
                    filler[fi]()
                    fi += 1
            while fi < n_f:
                filler[fi]()
                fi += 1

        # Q0/K0 first, then attention(g) zipped with projections(g+1).
        for u in emit_proj_units(0):
            u()
        for g in range(3):
            zip_emit(att_iters_for_pair(g), emit_proj_units(g + 1))

        # xT and the projection weights are dead now; free them (they live
        # on the right-side SBUF stack, popped LIFO: wp then xtp) before the
        # out-projection pools open so the SBUF budget holds.
        w_cm.__exit__(None, None, None)
        xt_cm.__exit__(None, None, None)

        wo_pool = es.enter_context(tc.tile_pool(name="wo", bufs=4))
        os_pool = es.enter_context(tc.tile_pool(name="os", bufs=4))
        wo_t = []
        for fc in range(4):
            w_t = wo_pool.tile([128, D], F32R, name=f"wo{fc}", tag=f"wo{fc}")
            nc.sync.dma_start(
                out=w_t,
                in_=wo_d[fc * 128:(fc + 1) * 128, :].bitcast(F32R))
            wo_t.append(w_t)

        def emit_outproj(j):
            for mt in range(4 * j, 4 * j + 4):
                msl = slice((mt - 4 * j) * 128, (mt - 4 * j) * 128 + 128)
                for nck in range(2):
                    op = pp_pool.tile([128, 512], F32,
                                      name=f"op{mt}_{nck}", tag="pp")
                    for g in range(4):
                        nc.tensor.matmul(
                            op[:],
                            ot_map[(j, g)][:, msl],
                            wo_t[g][:, nck * 512:(nck + 1) * 512],
                            start=(g == 0), stop=(g == 3))
                    osb = os_pool.tile([128, 512], F32,
                                       name=f"os{mt}_{nck}", tag="os")
                    nc.vector.tensor_copy(osb[:], op[:])
                    nc.sync.dma_start(
                        out=part_d[mt * 128:(mt + 1) * 128,
                                   nck * 512:(nck + 1) * 512],
                        in_=osb[:])

        # Last pair: out-projection for chunk j follows attention(j).
        g = 3
        for j in range(4):
            t_max = 4 * (j + 1)
            ot_map[("ps", g, j)] = [
                otp_pool.tile([65, 512], F32,
                              name=f"o{j}_{g}_{hl}", tag="ops")
                for hl in range(2)]
            for t in range(t_max):
                emit_att_iter(g, j, t, t_max)
            emit_normalize(g, j)
            emit_outproj(j)


def _get_program():
    global _PROGRAM
    if _PROGRAM is None:
        _PROGRAM = _build_program()
    return _PROGRAM


_EXEC = None


def _get_executor():
    """Build the sharded PJRT executable once and reuse it across calls.

    Mirrors bass2jax.run_bass_via_pjrt's multi-core branch, but caches the
    jitted callable so repeat kernel() calls skip retracing/recompilation.
    Returns (fn, in_names, out_names, out_shapes). fn takes globally
    concatenated inputs (n_cores*dim0, ...) plus donated zero output
    buffers, and returns concatenated outputs.
    """
    global _EXEC
    if _EXEC is None:
        import jax
        from jax.experimental.shard_map import shard_map
        from jax.sharding import Mesh, PartitionSpec

        from concourse import bass2jax

        bass2jax.install_neuronx_cc_hook()
        nc = _get_program()
        part_name = (nc.partition_id_tensor.name
                     if nc.partition_id_tensor else None)
        in_names, out_names, out_avals = [], [], []
        for alloc in nc.m.functions[0].allocations:
            if not isinstance(alloc, mybir.MemoryLocationSet):
                continue
            name = alloc.memorylocations[0].name
            if alloc.kind == "ExternalInput":
                if name != part_name:
                    in_names.append(name)
            elif alloc.kind == "ExternalOutput":
                out_names.append(name)
                out_avals.append(jax.core.ShapedArray(
                    tuple(alloc.tensor_shape), mybir.dt.np(alloc.dtype)))
        n_params = len(in_names)
        all_in = tuple(in_names) + tuple(out_names)
        if part_name is not None:
            all_in = all_in + (part_name,)

        def _body(*args):
            operands = list(args)
            if part_name is not None:
                operands.append(bass2jax.partition_id_tensor())
            outs = bass2jax._bass_exec_p.bind(
                *operands,
                out_avals=tuple(out_avals),
                in_names=all_in,
                out_names=tuple(out_names),
                lowering_input_output_aliases=(),
                sim_require_finite=True,
                sim_require_nnan=True,
                nc=nc)
            return tuple(outs)

        devices = jax.devices()[:N_CORES]
        mesh = Mesh(np.asarray(devices), ("core",))
        n_bufs = n_params + len(out_names)
        mapped = shard_map(_body, mesh=mesh,
                           in_specs=(PartitionSpec("core"),) * n_bufs,
                           out_specs=(PartitionSpec("core"),) * len(out_names),
                           check_rep=False)
        fn = jax.jit(mapped,
                     donate_argnums=tuple(range(n_params, n_bufs)),
                     keep_unused=True)
        # Non-donating twin: lets a timing loop reuse device-resident
        # argument buffers across calls (we write every element of every
        # output, so uninitialized result buffers are fine).
        fn_nodonate = jax.jit(mapped, keep_unused=True)
        out_shapes = [tuple(a.shape) for a in out_avals]
        _EXEC = (fn, fn_nodonate, in_names, out_names, out_shapes, mesh)
    return _EXEC


def run_cores(in_maps):
    """Run the SPMD program on 8 cores via the cached executable."""
    fn, _, in_names, out_names, out_shapes = _get_executor()[:5]
    concat_in = [np.concatenate([in_maps[c][n] for c in range(N_CORES)],
                                axis=0) for n in in_names]
    zeros = [np.zeros((N_CORES * s[0],) + s[1:], np.float32)
             for s in out_shapes]
    outs = fn(*concat_in, *zeros)
    res = []
    for c in range(N_CORES):
        res.append({
            n: np.asarray(outs[i]).reshape((N_CORES,) + out_shapes[i])[c]
            for i, n in enumerate(out_names)})
    return res


def make_in_maps(x, w_q, b_q, w_k, b_k, w_v, b_v, w_o, b_o):
    in_maps = []
    for c in range(N_CORES):
        b, hh = divmod(c, 2)
        cols = slice(hh * DH, (hh + 1) * DH)
        in_maps.append({
            "x": np.ascontiguousarray(x[b]),
            "wq": np.ascontiguousarray(w_q[:, cols]),
            "wk": np.ascontiguousarray(w_k[:, cols]),
            "wv": np.ascontiguousarray(w_v[:, cols]),
            "wo": np.ascontiguousarray(w_o[cols, :]),
            "bq": np.ascontiguousarray(b_q[cols]),
            "bk": np.ascontiguousarray(b_k[cols]),
        })
    return in_maps


def combine(parts, b_v, w_o, b_o):
    corr = (b_v @ w_o + b_o).astype(np.float32)
    out = np.empty((4, S, D), dtype=np.float32)
    for b in range(4):
        out[b] = parts[2 * b] + parts[2 * b + 1] + corr
    return out


def kernel(x, w_q, b_q, w_k, b_k, w_v, b_v, w_o, b_o):
    x = np.asarray(x, dtype=np.float32)
    w_q = np.asarray(w_q, dtype=np.float32)
    b_q = np.asarray(b_q, dtype=np.float32)
    w_k = np.asarray(w_k, dtype=np.float32)
    b_k = np.asarray(b_k, dtype=np.float32)
    w_v = np.asarray(w_v, dtype=np.float32)
    b_v = np.asarray(b_v, dtype=np.float32)
    w_o = np.asarray(w_o, dtype=np.float32)
    b_o = np.asarray(b_o, dtype=np.float32)

    in_maps = make_in_maps(x, w_q, b_q, w_k, b_k, w_v, b_v, w_o, b_o)
    res = run_cores(in_maps)
    parts = [res[c]["part"] for c in range(N_CORES)]
    return combine(parts, b_v, w_o, b_o)
